# revision 21
# baseline (speedup 1.0000x reference)
"""Causal single-head attention on 8 Trainium2 NeuronCores.

Problem: x[4, 4096, 1024], Wq/Wk/Wv[1024, 64] ->
  out = softmax(causal(Q K^T / 8)) V   per batch, fp32.

Sharding: core i handles batch b = i//2 with query-chunk parity p = i%2
(512-wide query chunks; core p owns global chunks {p, 2+p, 4+p, 6+p}).
Both cores of a pair load the full x[b] (transposed on host to [C, T]) and
compute full K/V; causal work is balanced by interleaving query chunks.

All matmul operands are bf16 (fp32 accumulate in PSUM): halves HBM traffic,
runs the PE at 1 cycle/row (the fp32 path is 4), and enables FWL weight
loads. Scores are computed transposed (S^T[k, q]) with contraction HS=64,
so two key tiles are packed concurrently in the PE array via partition
row-groups (tiles at base partition 0 and 64). exp runs once per key-tile
pair over [128, 1024] spanning two PSUM banks. Causality is applied as a
multiplicative 0/1 bf16 mask after exp (host-built maskP gives both pair
halves in one strided access). V carries a ones column so softmax row-sums
accumulate in the same PSUM tile as P@V; the unnormalized out^T [65, q] is
DMA'd out and the division + transpose happen on host.
"""

import numpy as np
import ml_dtypes

import concourse.bacc as bacc
import concourse.mybir as mybir
import concourse.tile as tile
from concourse.bass_utils import run_bass_kernel_spmd

# Problem dims
B, T, C, HS = 4, 4096, 1024, 64
P = 128           # partitions
CH = 512          # query-chunk width
CHP = 2 * CH      # chunk-pair width (one DMA)
NCH = T // CH     # 8 chunks
NSLOT = NCH // 2  # 4 local query slots per core
CSUB = C // P     # 8 contraction subtiles
NKT = T // P      # 32 key tiles total
NPAIR = NKT // 2  # 16 key-tile pairs
MASK_W = 896 + CH  # mask window width

BF16 = mybir.dt.bfloat16


def _build_program():
    nc = bacc.Bacc("TRN2")
    f32 = mybir.dt.float32
    EXP = mybir.ActivationFunctionType.Exp

    # x pre-transposed AND pre-tiled on host: xh[cp, ci, co, u] so each
    # chunk-pair DMA reads 16 KB contiguous per partition (full HBM BW,
    # ~128 descriptors instead of 2048). x0 duplicates chunk 0 so the
    # first projection can start after a half-size load.
    xh = nc.dram_tensor("xh", [NSLOT, P, CSUB, CHP], BF16, kind="ExternalInput").ap()
    x0 = nc.dram_tensor("x0", [P, CSUB, CH], BF16, kind="ExternalInput").ap()
    wqk = nc.dram_tensor("wqk", [C, 2 * HS], BF16, kind="ExternalInput").ap()
    wv = nc.dram_tensor("wv", [C, HS], BF16, kind="ExternalInput").ap()
    maskp_d = nc.dram_tensor("maskp", [P, 2, MASK_W], BF16, kind="ExternalInput").ap()
    pmask_d = nc.dram_tensor("pmask", [HS, CH], mybir.dt.uint8, kind="ExternalInput").ap()
    ident_d = nc.dram_tensor("ident", [HS, HS], BF16, kind="ExternalInput").ap()
    out_d = nc.dram_tensor("out", [HS + 1, NSLOT * CH], f32, kind="ExternalOutput").ap()

    wqk_r = wqk.rearrange("(co ci) m -> ci co m", ci=P)    # [128, 8, 128]
    wv_r = wv.rearrange("(co ci) m -> ci co m", ci=P)      # [128, 8, 64]

    with tile.TileContext(nc) as tc:
        with (
            tc.tile_pool(name="const", bufs=1) as const_pool,
            tc.tile_pool(name="persist", bufs=1) as persist,
            tc.tile_pool(name="xin", bufs=3) as xpool,
            tc.tile_pool(name="x0in", bufs=1) as x0pool,
            tc.tile_pool(name="vt", bufs=4) as vt_pool,
            tc.tile_pool(name="pt", bufs=3) as pt_pool,
            tc.tile_pool(name="osb", bufs=2) as osb_pool,
            tc.tile_pool(name="proj_ps", bufs=2, space="PSUM") as proj_ps,
            tc.tile_pool(name="st_ps", bufs=2, space="PSUM") as st_ps,
            tc.tile_pool(name="ot_ps", bufs=2, space="PSUM") as ot_ps,
        ):
            # ---- constants / persistent state ----
            # const loads go on the (otherwise idle) GpSimd queue so they
            # don't serialize ahead of the first x chunk on the Sync queue
            wqk_sb = const_pool.tile([P, CSUB, 2 * HS], BF16)
            wv_sb = const_pool.tile([P, CSUB, HS], BF16)
            maskp_sb = const_pool.tile([P, 2, MASK_W], BF16)
            pmask_sb = const_pool.tile([HS, CH], mybir.dt.uint8)
            ident_sb = const_pool.tile([HS, HS], BF16)
            nc.gpsimd.dma_start(wqk_sb[:], wqk_r)
            nc.gpsimd.dma_start(wv_sb[:], wv_r)
            nc.gpsimd.dma_start(maskp_sb[:], maskp_d)
            nc.gpsimd.dma_start(pmask_sb[:], pmask_d)
            nc.gpsimd.dma_start(ident_sb[:], ident_d)

            # K^T pairs: [0:64, u, :] = tile 2u, [64:128, u, :] = tile 2u+1
            kt_all = persist.tile([P, NPAIR, P], BF16)
            qt_stage = persist.tile([HS, NSLOT, CH], f32)    # Q^T select staging
            qt_slot = persist.tile([P, NSLOT, CH], BF16)     # Q^T dup'd both halves
            v_all = persist.tile([P, NKT, HS + 1], BF16)     # V with ones column
            nc.vector.memset(
                v_all[:, :, HS : HS + 1].bitcast(mybir.dt.uint16), 0x3F80
            )

            # ---- streamed projection + attention ----
            # first chunk arrives as 8 per-cs slices so the first projection
            # matmul only waits for 128 KB, not the full chunk
            x0_sb = x0pool.tile([P, CSUB, CH], BF16, tag="x0")
            for cs in range(CSUB):
                nc.sync.dma_start(x0_sb[:, cs, :], x0[:, cs, :])

            for cp in range(NSLOT):  # chunk pairs (even, odd)
                xc = xpool.tile([P, CSUB, CHP], BF16, tag="xc")
                # alternate hardware DMA queues (Sync vs Activation) so the
                # x stream isn't serialized through one queue; split per-cs
                # to keep descriptors at 2 KB (smaller descriptors pipeline
                # across the 16 DMA engines much better than 16 KB ones)
                eng = nc.sync if cp % 2 == 0 else nc.scalar
                for cs in range(CSUB):
                    eng.dma_start(xc[:, cs, :], xh[cp, :, cs, :])

                for half in range(2):
                    c = 2 * cp + half
                    lo = half * CH
                    xsrc = x0_sb if c == 0 else xc
                    xlo = 0 if c == 0 else lo
                    # Q^T (rows 0:64) and K^T (rows 64:128), stacked projection
                    qk_ps = proj_ps.tile([P, CH], f32, tag="proj")
                    for cs in range(CSUB):
                        nc.tensor.matmul(
                            qk_ps[:],
                            lhsT=wqk_sb[:, cs, :],
                            rhs=xsrc[:, cs, xlo : xlo + CH],
                            start=(cs == 0),
                            stop=(cs == CSUB - 1),
                        )
                    # chunk c holds key tiles 4c..4c+3 = pairs 2c, 2c+1
                    ksrc = qk_ps[HS:P, :].rearrange(
                        "p (i par c) -> p i par c", i=2, par=2, c=P
                    )
                    nc.vector.tensor_copy(
                        kt_all[0:HS, 2 * c : 2 * c + 2, :], ksrc[:, :, 0, :]
                    )
                    nc.vector.tensor_copy(
                        kt_all[HS:P, 2 * c : 2 * c + 2, :], ksrc[:, :, 1, :]
                    )
                    if half == 0:
                        nc.vector.tensor_copy(qt_stage[:, cp, :], qk_ps[0:HS, :])
                    else:
                        nc.vector.copy_predicated(
                            qt_stage[:, cp, :], pmask_sb[:], qk_ps[0:HS, :]
                        )
                        nc.vector.tensor_copy(qt_slot[0:HS, cp, :], qt_stage[:, cp, :])
                        nc.vector.tensor_copy(qt_slot[HS:P, cp, :], qt_stage[:, cp, :])

                    # V^T via wide matmuls (wv stationary), then PE-transpose
                    # each 128-block back to natural [t, h] layout
                    v_ps = proj_ps.tile([HS, CH], f32, tag="proj")
                    for cs in range(CSUB):
                        nc.tensor.matmul(
                            v_ps[:],
                            lhsT=wv_sb[:, cs, :],
                            rhs=xsrc[:, cs, xlo : xlo + CH],
                            start=(cs == 0),
                            stop=(cs == CSUB - 1),
                        )
                    vt_sb = vt_pool.tile([HS, CH], BF16, tag="vt")
                    nc.vector.tensor_copy(vt_sb[:], v_ps[:])
                    for tt in range(4):
                        tr = proj_ps.tile([P, HS], BF16, tag="proj")
                        nc.tensor.transpose(
                            tr[:], vt_sb[:, tt * P : (tt + 1) * P], ident_sb[:]
                        )
                        nc.vector.tensor_copy(v_all[:, 4 * c + tt, 0:HS], tr[:])

                # slot j = cp owns global chunk 2j+p; flush its attention row
                j = cp
                npair = 4 * j + 4
                ot = ot_ps.tile([P, CH], f32, tag="ot")
                for u in range(npair):
                    st = st_ps.tile([P, 2, CH], f32, tag="st")
                    # half 0 <- key tile 2u+1 (base partition 64),
                    # half 1 <- key tile 2u (base partition 0)
                    nc.tensor.matmul(
                        st[:, 0, :],
                        lhsT=kt_all[HS:P, u, :],
                        rhs=qt_slot[HS:P, j, :],
                        start=True,
                        stop=True,
                    )
                    nc.tensor.matmul(
                        st[:, 1, :],
                        lhsT=kt_all[0:HS, u, :],
                        rhs=qt_slot[0:HS, j, :],
                        start=True,
                        stop=True,
                    )
                    pt = pt_pool.tile([P, 2, CH], BF16, tag="pt")
                    nc.scalar.activation(pt[:], st[:], EXP, scale=float(HS) ** -0.5)
                    if u >= 4 * j:  # masked band of this slot
                        s2 = P * (8 * j + 6 - 2 * u)
                        nc.vector.tensor_mul(
                            pt[:], pt[:], maskp_sb[:, :, s2 : s2 + CH]
                        )
                    nc.tensor.matmul(
                        ot[0 : HS + 1, :],
                        lhsT=v_all[:, 2 * u + 1, :],
                        rhs=pt[:, 0, :],
                        start=(u == 0),
                        stop=False,
                    )
                    nc.tensor.matmul(
                        ot[0 : HS + 1, :],
                        lhsT=v_all[:, 2 * u, :],
                        rhs=pt[:, 1, :],
                        start=False,
                        stop=(u == npair - 1),
                    )

                # store unnormalized out^T + sums row; host divides/transposes
                o_sb = osb_pool.tile([HS + 1, CH], f32, tag="osb")
                nc.vector.tensor_copy(o_sb[:], ot[0 : HS + 1, :])
                nc.sync.dma_start(out_d[:, j * CH : (j + 1) * CH], o_sb[:])

    nc.compile()
    return nc


_CACHE = {}


def _get_program():
    if "nc" not in _CACHE:
        _CACHE["nc"] = _build_program()
    return _CACHE["nc"]


def _host_inputs(x, Wk, Wq, Wv):
    bf = ml_dtypes.bfloat16
    x = np.asarray(x, dtype=np.float32)
    wqk = np.ascontiguousarray(
        np.concatenate([np.asarray(Wq), np.asarray(Wk)], axis=1), dtype=np.float32
    ).astype(bf)
    wv = np.ascontiguousarray(np.asarray(Wv), dtype=np.float32).astype(bf)

    # xh[cp, ci, co, u] = x[b, cp*1024+u, co*128+ci]; x0 = chunk 0 alone
    xhs, x0s = [], []
    for b in range(B):
        v = x[b].reshape(NSLOT, CHP, CSUB, P)          # [cp, u, co, ci]
        xh = np.ascontiguousarray(v.transpose(0, 3, 2, 1)).astype(bf)
        xhs.append(xh)
        x0s.append(np.ascontiguousarray(xh[0, :, :, 0:CH]))

    # maskp[i, h, c] = 1 if (c + 128 h) >= i + (896 - 512 p) else 0
    ii = np.arange(P)[:, None, None]
    hh = np.arange(2)[None, :, None]
    cc = np.arange(MASK_W)[None, None, :]
    maskps = [
        ((cc + P * hh) >= (ii + (896 - 512 * p))).astype(bf) for p in range(2)
    ]
    pmasks = [np.full((HS, CH), p, dtype=np.uint8) for p in range(2)]
    ident = np.eye(HS, dtype=np.float32).astype(bf)

    in_maps = []
    for core in range(2 * B):
        b, p = core // 2, core % 2
        in_maps.append(
            {
                "xh": xhs[b],
                "x0": x0s[b],
                "wqk": wqk,
                "wv": wv,
                "maskp": maskps[p],
                "pmask": pmasks[p],
                "ident": ident,
            }
        )
    return in_maps


def _assemble(results):
    out = np.empty((B, T, HS), dtype=np.float32)
    for core in range(2 * B):
        b, p = core // 2, core % 2
        oc = np.asarray(results[core]["out"], dtype=np.float32)  # [65, 2048]
        for j in range(NSLOT):
            g = 2 * j + p
            blk = oc[:, j * CH : (j + 1) * CH]
            out[b, g * CH : (g + 1) * CH, :] = (blk[0:HS] / blk[HS : HS + 1]).T
    return out


def run(x, Wk, Wq, Wv, trace=False):
    nc = _get_program()
    in_maps = _host_inputs(x, Wk, Wq, Wv)
    res = run_bass_kernel_spmd(nc, in_maps, list(range(2 * B)), trace=trace)
    return _assemble(res.results), res


def kernel(x, Wk, Wq, Wv):
    out, _ = run(x, Wk, Wq, Wv)
    return out


# revision 22
# speedup vs baseline: 1.1949x; 1.1949x over previous
"""Causal single-head attention on 8 Trainium2 NeuronCores.

Problem: x[4, 4096, 1024], Wq/Wk/Wv[1024, 64] ->
  out = softmax(causal(Q K^T / 8)) V   per batch, fp32.

Sharding: core i handles batch b = i//2 with query-chunk parity p = i%2
(512-wide query chunks; core p owns global chunks {p, 2+p, 4+p, 6+p}).
Both cores of a pair load the full x[b] (transposed on host to [C, T]) and
compute full K/V; causal work is balanced by interleaving query chunks.

All matmul operands are bf16 (fp32 accumulate in PSUM): halves HBM traffic,
runs the PE at 1 cycle/row (the fp32 path is 4), and enables FWL weight
loads. Scores are computed transposed (S^T[k, q]) with contraction HS=64,
so two key tiles are packed concurrently in the PE array via partition
row-groups (tiles at base partition 0 and 64). exp runs once per key-tile
pair over [128, 1024] spanning two PSUM banks. Causality is applied as a
multiplicative 0/1 bf16 mask after exp (host-built maskP gives both pair
halves in one strided access). V carries a ones column so softmax row-sums
accumulate in the same PSUM tile as P@V; the unnormalized out^T [65, q] is
DMA'd out and the division + transpose happen on host.
"""

import numpy as np
import ml_dtypes

import concourse.bacc as bacc
import concourse.mybir as mybir
import concourse.tile as tile
from concourse.bass_utils import run_bass_kernel_spmd

# Problem dims
B, T, C, HS = 4, 4096, 1024, 64
P = 128           # partitions
CH = 512          # query-chunk width
CHP = 2 * CH      # chunk-pair width (one DMA)
NCH = T // CH     # 8 chunks
NSLOT = NCH // 2  # 4 local query slots per core
CSUB = C // P     # 8 contraction subtiles
NKT = T // P      # 32 key tiles total
NPAIR = NKT // 2  # 16 key-tile pairs
MASK_W = 896 + CH  # mask window width

BF16 = mybir.dt.bfloat16


def _build_program():
    nc = bacc.Bacc("TRN2")
    f32 = mybir.dt.float32
    EXP = mybir.ActivationFunctionType.Exp

    # x transposed on host to [C, T]; x0 duplicates chunk 0 in a
    # contiguous per-partition layout so the first projection can start
    # after a single 128 KB slice lands.
    xT = nc.dram_tensor("xT", [C, T], BF16, kind="ExternalInput").ap()
    x0 = nc.dram_tensor("x0", [P, CSUB, CH], BF16, kind="ExternalInput").ap()
    wqk = nc.dram_tensor("wqk", [C, 2 * HS], BF16, kind="ExternalInput").ap()
    wv = nc.dram_tensor("wv", [C, HS], BF16, kind="ExternalInput").ap()
    maskp_d = nc.dram_tensor("maskp", [P, 2, MASK_W], BF16, kind="ExternalInput").ap()
    pmask_d = nc.dram_tensor("pmask", [HS, CH], mybir.dt.uint8, kind="ExternalInput").ap()
    ident_d = nc.dram_tensor("ident", [HS, HS], BF16, kind="ExternalInput").ap()
    out_d = nc.dram_tensor("out", [HS + 1, NSLOT * CH], f32, kind="ExternalOutput").ap()

    xT_r = xT.rearrange("(co ci) t -> ci co t", ci=P)      # [128, 8, 4096]
    wqk_r = wqk.rearrange("(co ci) m -> ci co m", ci=P)    # [128, 8, 128]
    wv_r = wv.rearrange("(co ci) m -> ci co m", ci=P)      # [128, 8, 64]

    with tile.TileContext(nc) as tc:
        with (
            tc.tile_pool(name="const", bufs=1) as const_pool,
            tc.tile_pool(name="persist", bufs=1) as persist,
            tc.tile_pool(name="xin", bufs=3) as xpool,
            tc.tile_pool(name="x0in", bufs=1) as x0pool,
            tc.tile_pool(name="vt", bufs=4) as vt_pool,
            tc.tile_pool(name="pt", bufs=3) as pt_pool,
            tc.tile_pool(name="osb", bufs=2) as osb_pool,
            tc.tile_pool(name="proj_ps", bufs=2, space="PSUM") as proj_ps,
            tc.tile_pool(name="st_ps", bufs=2, space="PSUM") as st_ps,
            tc.tile_pool(name="ot_ps", bufs=2, space="PSUM") as ot_ps,
        ):
            # ---- constants / persistent state ----
            # const loads go on the (otherwise idle) GpSimd queue so they
            # don't serialize ahead of the first x chunk on the Sync queue
            wqk_sb = const_pool.tile([P, CSUB, 2 * HS], BF16)
            wv_sb = const_pool.tile([P, CSUB, HS], BF16)
            maskp_sb = const_pool.tile([P, 2, MASK_W], BF16)
            pmask_sb = const_pool.tile([HS, CH], mybir.dt.uint8)
            ident_sb = const_pool.tile([HS, HS], BF16)
            nc.gpsimd.dma_start(wqk_sb[:], wqk_r)
            nc.gpsimd.dma_start(wv_sb[:], wv_r)
            nc.gpsimd.dma_start(maskp_sb[:], maskp_d)
            nc.gpsimd.dma_start(pmask_sb[:], pmask_d)
            nc.gpsimd.dma_start(ident_sb[:], ident_d)

            # K^T pairs: [0:64, u, :] = tile 2u, [64:128, u, :] = tile 2u+1
            kt_all = persist.tile([P, NPAIR, P], BF16)
            qt_stage = persist.tile([HS, NSLOT, CH], f32)    # Q^T select staging
            qt_slot = persist.tile([P, NSLOT, CH], BF16)     # Q^T dup'd both halves
            v_all = persist.tile([P, NKT, HS + 1], BF16)     # V with ones column
            nc.vector.memset(
                v_all[:, :, HS : HS + 1].bitcast(mybir.dt.uint16), 0x3F80
            )

            # ---- streamed projection + attention ----
            # first chunk arrives as 8 per-cs slices so the first projection
            # matmul only waits for 128 KB, not the full chunk
            x0_sb = x0pool.tile([P, CSUB, CH], BF16, tag="x0")
            for cs in range(CSUB):
                nc.sync.dma_start(x0_sb[:, cs, :], x0[:, cs, :])

            for cp in range(NSLOT):  # chunk pairs (even, odd)
                xc = xpool.tile([P, CSUB, CHP], BF16, tag="xc")
                nc.sync.dma_start(xc[:], xT_r[:, :, cp * CHP : (cp + 1) * CHP])

                for half in range(2):
                    c = 2 * cp + half
                    lo = half * CH
                    xsrc = x0_sb if c == 0 else xc
                    xlo = 0 if c == 0 else lo
                    # Q^T (rows 0:64) and K^T (rows 64:128), stacked projection
                    qk_ps = proj_ps.tile([P, CH], f32, tag="proj")
                    for cs in range(CSUB):
                        nc.tensor.matmul(
                            qk_ps[:],
                            lhsT=wqk_sb[:, cs, :],
                            rhs=xsrc[:, cs, xlo : xlo + CH],
                            start=(cs == 0),
                            stop=(cs == CSUB - 1),
                        )
                    # chunk c holds key tiles 4c..4c+3 = pairs 2c, 2c+1
                    ksrc = qk_ps[HS:P, :].rearrange(
                        "p (i par c) -> p i par c", i=2, par=2, c=P
                    )
                    nc.vector.tensor_copy(
                        kt_all[0:HS, 2 * c : 2 * c + 2, :], ksrc[:, :, 0, :]
                    )
                    nc.vector.tensor_copy(
                        kt_all[HS:P, 2 * c : 2 * c + 2, :], ksrc[:, :, 1, :]
                    )
                    if half == 0:
                        nc.vector.tensor_copy(qt_stage[:, cp, :], qk_ps[0:HS, :])
                    else:
                        nc.vector.copy_predicated(
                            qt_stage[:, cp, :], pmask_sb[:], qk_ps[0:HS, :]
                        )
                        nc.vector.tensor_copy(qt_slot[0:HS, cp, :], qt_stage[:, cp, :])
                        nc.vector.tensor_copy(qt_slot[HS:P, cp, :], qt_stage[:, cp, :])

                    # V natural ([t, h]) via x^T blocks as stationary operand
                    v_ps = proj_ps.tile([P, 4, HS], f32, tag="proj")
                    for tt in range(4):
                        for cs in range(CSUB):
                            nc.tensor.matmul(
                                v_ps[:, tt, :],
                                lhsT=xsrc[:, cs, xlo + tt * P : xlo + (tt + 1) * P],
                                rhs=wv_sb[:, cs, :],
                                start=(cs == 0),
                                stop=(cs == CSUB - 1),
                            )
                    nc.vector.tensor_copy(
                        v_all[:, 4 * c : 4 * c + 4, 0:HS], v_ps[:]
                    )

                # slot j = cp owns global chunk 2j+p; flush its attention row
                j = cp
                npair = 4 * j + 4
                ot = ot_ps.tile([P, CH], f32, tag="ot")
                for u in range(npair):
                    st = st_ps.tile([P, 2, CH], f32, tag="st")
                    # half 0 <- key tile 2u+1 (base partition 64),
                    # half 1 <- key tile 2u (base partition 0)
                    nc.tensor.matmul(
                        st[:, 0, :],
                        lhsT=kt_all[HS:P, u, :],
                        rhs=qt_slot[HS:P, j, :],
                        start=True,
                        stop=True,
                    )
                    nc.tensor.matmul(
                        st[:, 1, :],
                        lhsT=kt_all[0:HS, u, :],
                        rhs=qt_slot[0:HS, j, :],
                        start=True,
                        stop=True,
                    )
                    pt = pt_pool.tile([P, 2, CH], BF16, tag="pt")
                    nc.scalar.activation(pt[:], st[:], EXP, scale=float(HS) ** -0.5)
                    if u >= 4 * j:  # masked band of this slot
                        s2 = P * (8 * j + 6 - 2 * u)
                        nc.vector.tensor_mul(
                            pt[:], pt[:], maskp_sb[:, :, s2 : s2 + CH]
                        )
                    nc.tensor.matmul(
                        ot[0 : HS + 1, :],
                        lhsT=v_all[:, 2 * u + 1, :],
                        rhs=pt[:, 0, :],
                        start=(u == 0),
                        stop=False,
                    )
                    nc.tensor.matmul(
                        ot[0 : HS + 1, :],
                        lhsT=v_all[:, 2 * u, :],
                        rhs=pt[:, 1, :],
                        start=False,
                        stop=(u == npair - 1),
                    )

                # store unnormalized out^T + sums row; host divides/transposes
                o_sb = osb_pool.tile([HS + 1, CH], f32, tag="osb")
                nc.vector.tensor_copy(o_sb[:], ot[0 : HS + 1, :])
                nc.sync.dma_start(out_d[:, j * CH : (j + 1) * CH], o_sb[:])

    nc.compile()
    return nc


_CACHE = {}


def _get_program():
    if "nc" not in _CACHE:
        _CACHE["nc"] = _build_program()
    return _CACHE["nc"]


def _host_inputs(x, Wk, Wq, Wv):
    bf = ml_dtypes.bfloat16
    x = np.asarray(x, dtype=np.float32)
    wqk = np.ascontiguousarray(
        np.concatenate([np.asarray(Wq), np.asarray(Wk)], axis=1), dtype=np.float32
    ).astype(bf)
    wv = np.ascontiguousarray(np.asarray(Wv), dtype=np.float32).astype(bf)

    xTs, x0s = [], []
    for b in range(B):
        xT = np.ascontiguousarray(x[b].T).astype(bf)   # [C, T]
        xTs.append(xT)
        # x0[ci, co, u] = xT[co*128+ci, u]
        v = xT[:, 0:CH].reshape(CSUB, P, CH)
        x0s.append(np.ascontiguousarray(v.transpose(1, 0, 2)))

    # maskp[i, h, c] = 1 if (c + 128 h) >= i + (896 - 512 p) else 0
    ii = np.arange(P)[:, None, None]
    hh = np.arange(2)[None, :, None]
    cc = np.arange(MASK_W)[None, None, :]
    maskps = [
        ((cc + P * hh) >= (ii + (896 - 512 * p))).astype(bf) for p in range(2)
    ]
    pmasks = [np.full((HS, CH), p, dtype=np.uint8) for p in range(2)]
    ident = np.eye(HS, dtype=np.float32).astype(bf)

    in_maps = []
    for core in range(2 * B):
        b, p = core // 2, core % 2
        in_maps.append(
            {
                "xT": xTs[b],
                "x0": x0s[b],
                "wqk": wqk,
                "wv": wv,
                "maskp": maskps[p],
                "pmask": pmasks[p],
                "ident": ident,
            }
        )
    return in_maps


def _assemble(results):
    out = np.empty((B, T, HS), dtype=np.float32)
    for core in range(2 * B):
        b, p = core // 2, core % 2
        oc = np.asarray(results[core]["out"], dtype=np.float32)  # [65, 2048]
        for j in range(NSLOT):
            g = 2 * j + p
            blk = oc[:, j * CH : (j + 1) * CH]
            out[b, g * CH : (g + 1) * CH, :] = (blk[0:HS] / blk[HS : HS + 1]).T
    return out


def run(x, Wk, Wq, Wv, trace=False):
    nc = _get_program()
    in_maps = _host_inputs(x, Wk, Wq, Wv)
    res = run_bass_kernel_spmd(nc, in_maps, list(range(2 * B)), trace=trace)
    return _assemble(res.results), res


def kernel(x, Wk, Wq, Wv):
    out, _ = run(x, Wk, Wq, Wv)
    return out


# revision 25
# speedup vs baseline: 1.2679x; 1.0610x over previous
"""Causal single-head attention on 8 Trainium2 NeuronCores.

Problem: x[4, 4096, 1024], Wq/Wk/Wv[1024, 64] ->
  out = softmax(causal(Q K^T / 8)) V   per batch, fp32.

Sharding: core i handles batch b = i//2 with query-chunk parity p = i%2 at
256-query granularity: core p owns global 256-chunks {2j+p : j=0..7}. Both
cores of a pair load the full x[b] (transposed on host to [C, T]) and
compute full K/V; causal work is balanced by interleaving query chunks.

All matmul operands are bf16 (fp32 accumulate in PSUM). Scores are computed
transposed (S^T[k, q]) with contraction HS=64, so two key tiles run
concurrently in the PE array via partition row-groups (base partition 0 and
64). Each flush step covers a "quad" (4 key tiles = 512 keys x 256 queries)
in one [128, 4, 256] PSUM tile: one exp ACTIVATE per quad, and causality is
one constant-mask bf16 multiply on the diagonal quad per slot. V carries a
ones column so softmax row-sums accumulate with P@V; the unnormalized
out^T [65, q] is DMA'd out and division + transpose happen on host.
"""

import numpy as np
import ml_dtypes

import concourse.bacc as bacc
import concourse.mybir as mybir
import concourse.tile as tile
from concourse.bass_utils import run_bass_kernel_spmd

# Problem dims
B, T, C, HS = 4, 4096, 1024, 64
P = 128           # partitions
CH = 512          # projection chunk width
CHP = 2 * CH      # chunk-pair width (one DMA)
CHA = 256         # attention query-slot width
NSLOT = 8         # query slots per core (256 wide)
CSUB = C // P     # 8 contraction subtiles
NKT = T // P      # 32 key tiles total
NPAIR = NKT // 2  # 16 key-tile pairs

BF16 = mybir.dt.bfloat16
# key tile (within quad) held by each st/pt slot: slot s <-> tile 4w+QORD[s]
QORD = (1, 3, 0, 2)


def _build_program():
    nc = bacc.Bacc("TRN2")
    f32 = mybir.dt.float32
    EXP = mybir.ActivationFunctionType.Exp

    xT = nc.dram_tensor("xT", [C, T], BF16, kind="ExternalInput").ap()
    x0 = nc.dram_tensor("x0", [P, CSUB, CH], BF16, kind="ExternalInput").ap()
    wqk = nc.dram_tensor("wqk", [C, 2 * HS], BF16, kind="ExternalInput").ap()
    wv = nc.dram_tensor("wv", [C, HS], BF16, kind="ExternalInput").ap()
    maskq_d = nc.dram_tensor("maskq", [P, 4, CHA], BF16, kind="ExternalInput").ap()
    pmask_d = nc.dram_tensor("pmask", [HS, CHA], mybir.dt.uint8, kind="ExternalInput").ap()
    out_d = nc.dram_tensor("out", [HS + 1, NSLOT * CHA], f32, kind="ExternalOutput").ap()

    xT_r = xT.rearrange("(co ci) t -> ci co t", ci=P)      # [128, 8, 4096]
    wqk_r = wqk.rearrange("(co ci) m -> ci co m", ci=P)    # [128, 8, 128]
    wv_r = wv.rearrange("(co ci) m -> ci co m", ci=P)      # [128, 8, 64]

    with tile.TileContext(nc) as tc:
        with (
            tc.tile_pool(name="const", bufs=1) as const_pool,
            tc.tile_pool(name="persist", bufs=1) as persist,
            tc.tile_pool(name="xin", bufs=3) as xpool,
            tc.tile_pool(name="x0in", bufs=1) as x0pool,
            tc.tile_pool(name="pt", bufs=3) as pt_pool,
            tc.tile_pool(name="osb", bufs=2) as osb_pool,
            tc.tile_pool(name="proj_ps", bufs=2, space="PSUM") as proj_ps,
            tc.tile_pool(name="st_ps", bufs=2, space="PSUM") as st_ps,
            tc.tile_pool(name="ot_ps", bufs=2, space="PSUM") as ot_ps,
        ):
            # const loads go on the (otherwise idle) GpSimd queue so they
            # don't serialize ahead of the first x chunk on the Sync queue
            wqk_sb = const_pool.tile([P, CSUB, 2 * HS], BF16)
            wv_sb = const_pool.tile([P, CSUB, HS], BF16)
            maskq_sb = const_pool.tile([P, 4, CHA], BF16)
            pmask_sb = const_pool.tile([HS, CHA], mybir.dt.uint8)
            nc.gpsimd.dma_start(wqk_sb[:], wqk_r)
            nc.gpsimd.dma_start(wv_sb[:], wv_r)
            nc.gpsimd.dma_start(maskq_sb[:], maskq_d)
            nc.gpsimd.dma_start(pmask_sb[:], pmask_d)

            # K^T pairs: [0:64, u, :] = tile 2u, [64:128, u, :] = tile 2u+1
            kt_all = persist.tile([P, NPAIR, P], BF16)
            qt_slot = persist.tile([P, NSLOT, CHA], BF16)    # Q^T dup'd halves
            v_all = persist.tile([P, NKT, HS + 1], BF16)     # V with ones col
            nc.vector.memset(
                v_all[:, :, HS : HS + 1].bitcast(mybir.dt.uint16), 0x3F80
            )

            # first chunk arrives as 8 per-cs slices so the first projection
            # matmul only waits for 128 KB, not the full chunk
            x0_sb = x0pool.tile([P, CSUB, CH], BF16, tag="x0")
            for cs in range(CSUB):
                nc.sync.dma_start(x0_sb[:, cs, :], x0[:, cs, :])

            for cp in range(4):  # x chunk pairs
                xc = xpool.tile([P, CSUB, CHP], BF16, tag="xc")
                nc.sync.dma_start(xc[:], xT_r[:, :, cp * CHP : (cp + 1) * CHP])

                for half in range(2):
                    s = 2 * cp + half  # projection chunk = slot index
                    lo = half * CH
                    xsrc = x0_sb if s == 0 else xc
                    xlo = 0 if s == 0 else lo
                    # Q^T (rows 0:64) and K^T (rows 64:128), stacked
                    qk_ps = proj_ps.tile([P, CH], f32, tag="proj")
                    for cs in range(CSUB):
                        nc.tensor.matmul(
                            qk_ps[:],
                            lhsT=wqk_sb[:, cs, :],
                            rhs=xsrc[:, cs, xlo : xlo + CH],
                            start=(cs == 0),
                            stop=(cs == CSUB - 1),
                        )
                    # chunk s holds key tiles 4s..4s+3 = pairs 2s, 2s+1
                    ksrc = qk_ps[HS:P, :].rearrange(
                        "p (i par c) -> p i par c", i=2, par=2, c=P
                    )
                    nc.vector.tensor_copy(
                        kt_all[0:HS, 2 * s : 2 * s + 2, :], ksrc[:, :, 0, :]
                    )
                    nc.vector.tensor_copy(
                        kt_all[HS:P, 2 * s : 2 * s + 2, :], ksrc[:, :, 1, :]
                    )
                    # slot s owns 256-queries [512 s + 256 p, +256): select
                    # the matching half of this chunk's Q via the predicate
                    for hb in (0, HS):
                        nc.vector.tensor_copy(
                            qt_slot[hb : hb + HS, s, :], qk_ps[0:HS, 0:CHA]
                        )
                        nc.vector.copy_predicated(
                            qt_slot[hb : hb + HS, s, :],
                            pmask_sb[:],
                            qk_ps[0:HS, CHA:CH],
                        )

                    # V natural ([t, h]) via x^T blocks as stationary operand
                    v_ps = proj_ps.tile([P, 4, HS], f32, tag="proj")
                    for tt in range(4):
                        for cs in range(CSUB):
                            nc.tensor.matmul(
                                v_ps[:, tt, :],
                                lhsT=xsrc[:, cs, xlo + tt * P : xlo + (tt + 1) * P],
                                rhs=wv_sb[:, cs, :],
                                start=(cs == 0),
                                stop=(cs == CSUB - 1),
                            )
                    nc.vector.tensor_copy(
                        v_all[:, 4 * s : 4 * s + 4, 0:HS], v_ps[:]
                    )

                    # flush slot j = s: quads w = 0..j, each = key tiles
                    # 4w..4w+3 vs this slot's 256 queries
                    j = s
                    ot = ot_ps.tile([P, CHA], f32, tag="ot")
                    for w in range(j + 1):
                        st = st_ps.tile([P, 4, CHA], f32, tag="st")
                        # issue order alternates row groups for LDW overlap;
                        # concurrent matmuls (issues 0&1, 2&3) must hit
                        # DIFFERENT PSUM banks (a start=True bank-clear
                        # racing a concurrent drain corrupts the bank), so
                        # slots go A,B,A,B; start=True only on each bank's
                        # first write (slots 0 and 2), the second write
                        # lands on cleared has_written bits and overwrites
                        for issue, (slot, o) in enumerate(
                            ((0, 1), (2, 0), (1, 3), (3, 2))
                        ):
                            u, hi = divmod(o, 2)  # pair 2w+u, row half hi
                            hb = HS if hi else 0
                            nc.tensor.matmul(
                                st[:, slot, :],
                                lhsT=kt_all[hb : hb + HS, 2 * w + u, :],
                                rhs=qt_slot[hb : hb + HS, j, :],
                                start=(issue < 2),
                                stop=(issue >= 2),
                                skip_group_check=True,
                            )
                        pt = pt_pool.tile([P, 4, CHA], BF16, tag="pt")
                        nc.scalar.activation(
                            pt[:], st[:], EXP, scale=float(HS) ** -0.5
                        )
                        if w == j:  # diagonal quad: constant causal mask
                            nc.vector.tensor_mul(pt[:], pt[:], maskq_sb[:])
                        for slot, o in ((0, 1), (2, 0), (1, 3), (3, 2)):
                            nc.tensor.matmul(
                                ot[0 : HS + 1, :],
                                lhsT=v_all[:, 4 * w + o, :],
                                rhs=pt[:, slot, :],
                                start=(w == 0 and slot == 0),
                                stop=(w == j and slot == 3),
                            )

                    # store unnormalized out^T + sums row; host finishes
                    o_sb = osb_pool.tile([HS + 1, CHA], f32, tag="osb")
                    nc.vector.tensor_copy(o_sb[:], ot[0 : HS + 1, :])
                    nc.sync.dma_start(out_d[:, j * CHA : (j + 1) * CHA], o_sb[:])

    nc.compile()
    return nc


_CACHE = {}


def _get_program():
    if "nc" not in _CACHE:
        _CACHE["nc"] = _build_program()
    return _CACHE["nc"]


def _host_inputs(x, Wk, Wq, Wv):
    bf = ml_dtypes.bfloat16
    x = np.asarray(x, dtype=np.float32)
    wqk = np.ascontiguousarray(
        np.concatenate([np.asarray(Wq), np.asarray(Wk)], axis=1), dtype=np.float32
    ).astype(bf)
    wv = np.ascontiguousarray(np.asarray(Wv), dtype=np.float32).astype(bf)

    xTs, x0s = [], []
    for b in range(B):
        xT = np.ascontiguousarray(x[b].T).astype(bf)   # [C, T]
        xTs.append(xT)
        v = xT[:, 0:CH].reshape(CSUB, P, CH)
        x0s.append(np.ascontiguousarray(v.transpose(1, 0, 2)))

    # maskq[i, q, c] = 1 iff c >= 128*QORD[q] + i - 256 p   (diagonal quad)
    ii = np.arange(P)[:, None, None]
    qq = np.array(QORD)[None, :, None]
    cc = np.arange(CHA)[None, None, :]
    maskqs = [
        (cc >= (128 * qq + ii - 256 * p)).astype(bf) for p in range(2)
    ]
    pmasks = [np.full((HS, CHA), p, dtype=np.uint8) for p in range(2)]

    in_maps = []
    for core in range(2 * B):
        b, p = core // 2, core % 2
        in_maps.append(
            {
                "xT": xTs[b],
                "x0": x0s[b],
                "wqk": wqk,
                "wv": wv,
                "maskq": maskqs[p],
                "pmask": pmasks[p],
            }
        )
    return in_maps


def _assemble(results):
    out = np.empty((B, T, HS), dtype=np.float32)
    for core in range(2 * B):
        b, p = core // 2, core % 2
        oc = np.asarray(results[core]["out"], dtype=np.float32)  # [65, 2048]
        for j in range(NSLOT):
            g = 2 * j + p
            blk = oc[:, j * CHA : (j + 1) * CHA]
            out[b, g * CHA : (g + 1) * CHA, :] = (blk[0:HS] / blk[HS : HS + 1]).T
    return out


def run(x, Wk, Wq, Wv, trace=False):
    nc = _get_program()
    in_maps = _host_inputs(x, Wk, Wq, Wv)
    res = run_bass_kernel_spmd(nc, in_maps, list(range(2 * B)), trace=trace)
    return _assemble(res.results), res


def kernel(x, Wk, Wq, Wv):
    out, _ = run(x, Wk, Wq, Wv)
    return out
